# revision 32
# baseline (speedup 1.0000x reference)
"""Trainium2 Bass kernel for nn_Classifier_69818988363910 (segment_reduce).

Reference computation (after dead-code elimination):
    local = relu(x @ W1^T)                        # [60000, 2048]
    feats = local.reshape(2000, 30, 2048).mean(1) # [2000, 2048]
    logits = concat(feats, feats) @ Wlin^T        # [2000, 1000]
           = feats @ (Wlin[:, :2048] + Wlin[:, 2048:])^T
y / W2 are computed but unused in the reference (original-code bug), so the
output depends only on x, W1, Wlin.

Sharding: data-parallel over the 8 NeuronCores along T (7500 rows = 250
segments per core); W1 / Wc replicated. No collectives; host gathers.

Device kernel per core (fp32 accumulation in PSUM throughout):
    MM1 on PE:   z[e, t] = sum_d W1T[d, e] * xT[d, t]
                 bf16 mode: 8 k-tiles of 128;  fp8 mode: 4 DoubleRow
                 super-k-tiles of 256 (2x PE rate)
    relu on ACT: psum -> sbuf
    pool on DVE: tensor_reduce over [128, segs, 30] view (sum; the 1/30
                 mean scale and the fp8 W1 pre-scale are folded into Wc
                 on the host)
    MM2 on PE:   logits[s, c] = sum_e featsT[e, s] * WcT[e, c]  (bf16)

Perf notes (from trace analysis; baseline 240us -> ~237us):
  - The PE runs MM1 at the fp8-DoubleRow peak (202ns per 256x128x480
    matmul, 157 TF/s/core), so wall time = startup + 202.7us MM1 +
    13.5us MM2 + writeback + ~8us fixed framework teardown.
  - The Tensor engine p-state ramps 1.54 -> 0.83 -> 0.42 ns/cycle with
    continuous busy time; a cold start or mid-stream stall drops it
    back (~2us to recover).  Dummy warmup matmuls on a memset tile keep
    the PE busy from preamble-end until the first real weights land, so
    MM1 opens at full clock.
  - Startup is bound by the DMA descriptor generator (~240 B/ns,
    round-robin one DMA per HWDGE ring, ~12ns per descriptor, so only
    2KB+ descriptors reach full rate).  W1 streams as (kt, j) slabs
    (full rows of [D, E]: 2KB/partition contiguous both sides), j1 on
    sync / j0 on scalar in consumption order; chunk-0 x rides gpsimd
    (whose ring aggregates descriptors into ~6KB packets).  A
    1-descriptor micro-DMA per engine first: ring bring-up is
    serialized and costs ~5ns per descriptor of the ring's HEAD DMA.
  - Wc loads as ONE 4MB DMA on scalar AFTER the startup-critical pieces
    (it is only needed ~200us in, and would hog the fabric).
  - logits copy out as bf16 (half the bytes); each 125x500 quarter is
    DMA'd as soon as its copy lands, alternating gpsimd/scalar rings so
    completions overlap.  Host casts back to f32.
"""

import os

import numpy as np
import ml_dtypes

BF16 = ml_dtypes.bfloat16
FP8 = ml_dtypes.float8_e4m3

MODE = os.environ.get("BASS_KERNEL_MODE", "fp8")    # "bf16" | "fp8"
W1_SCALE = 32.0                                     # fp8 mode: keep W1 out of subnormals
N_WARM = int(os.environ.get("BASS_KERNEL_WARMUP", "22"))
WARM_W = 256                                        # warmup matmul moving dim

N_CORES = 8
T, D, E, C, J = 60000, 1024, 2048, 1000, 30
T_LOC = T // N_CORES          # 7500 rows per core
S_LOC = T_LOC // J            # 250 segments per core
CHUNK = 480                   # t-chunk (16 segments); last chunk is 300
E_TILES = E // 128            # 16
S_BLK = 125                   # MM2 output rows per block (2 blocks)
C_BLK = 500                   # MM2 output cols per chunk (2 chunks)

_cache = {}


def _build(mode):
    from concourse import bacc, mybir
    from concourse.tile import TileContext

    f32 = mybir.dt.float32
    bf16 = mybir.dt.bfloat16
    fp8 = mybir.dt.float8e4
    in_dt = fp8 if mode == "fp8" else bf16
    KT = 4 if mode == "fp8" else 8          # accumulation steps per psum group
    perf = mybir.MatmulPerfMode.DoubleRow if mode == "fp8" else None

    nc = bacc.Bacc(trn_type="TRN2", target_bir_lowering=False, debug=False,
                   num_devices=N_CORES, num_swdge_queues=4)

    # x shard pre-chunked on the host to [chunk][p=128][d_tile=8][t] so each
    # chunk is one partition-contiguous DMA (3840B descriptors instead of
    # 8 DMAs x 480B descriptors).  W1 likewise host-packed into
    # [kt][half][p][(j)][e] piece blocks so each (kt, half) piece is one
    # contiguous 2KB run per partition — the HWDGE queue head dispatches
    # descriptors serially (~12-18ns each), so 2KB descriptors move ~4x
    # the bytes/s of the 512B-1KB ones a [D, E] layout would give.
    xt_d = nc.declare_dram_parameter("xt", [D * T_LOC], in_dt, isOutput=False)
    w1t_d = nc.declare_dram_parameter("w1t", [D, E], in_dt, isOutput=False)
    wct_d = nc.declare_dram_parameter("wct", [E, C], bf16, isOutput=False)
    out_d = nc.declare_dram_parameter("out", [S_LOC, C], bf16, isOutput=True)

    # t-chunks: 15 x 480 + 1 x 300
    chunks = []
    t0 = 0
    while t0 < T_LOC:
        w = min(CHUNK, T_LOC - t0)
        chunks.append((t0, w))
        t0 += w

    wct_v = wct_d[:, :].rearrange("(e p) c -> p e c", p=128)
    h = E // 2

    with TileContext(nc) as tc:
        with (
            tc.tile_pool(name="xin", bufs=2) as px,
            tc.tile_pool(name="wgt", bufs=1) as pw,
            tc.tile_pool(name="zrl", bufs=4) as pz,
        ):
            # PE warmup fodder: the memset is gpsimd's first instruction, so
            # the dummy matmuls can start right at preamble-end.
            if mode == "fp8":
                dmy = pw.tile([128, 2, WARM_W], fp8, tag="dmy", name="dmy")
            else:
                dmy = pw.tile([128, WARM_W], bf16, tag="dmy", name="dmy")
            nc.gpsimd.memset(dmy, 0)

            # --- W1 tiles: the HWDGE queue head dispatches descriptors
            # serially (~12ns each), so startup weight latency is set by
            # descriptor SIZE and by how many pieces queue ahead.  Pieces
            # are k-tile halves (1KB descriptors in the [D, E] layout) and
            # ride TWO queue heads in parallel (scalar + sync), interleaved
            # with chunk 0's x k-pairs in exact consumption order:
            #   sync:   x(kt0) w(1,h0) x(kt1) w(3,h0) w(1,h1) w(3,h1) chunks
            #   scalar: w(0,h0) x(kt2) w(2,h0) x(kt3) w(0,h1) w(2,h1)
            # (chunk 0 runs two 8-e-tile waves, kt-outer: wave 1 reads the
            # h0 half of every k-tile, wave 2 the h1 half.)
            if mode == "fp8":
                w1_sb = [pw.tile([128, 2, E], fp8, tag=f"w1_{k}",
                                 name=f"w1_{k}") for k in range(KT)]
            else:
                w1_sb = [pw.tile([128, E], bf16, tag=f"w1_{k}",
                                 name=f"w1_{k}") for k in range(KT)]

            def wpiece(eng, kt, piece):
                if mode == "fp8":
                    # piece = j: one full 128-row j-slab, 2KB/partition
                    r0 = kt * 256 + piece * 128
                    eng.dma_start(out=w1_sb[kt][:, piece, :],
                                  in_=w1t_d[r0:r0 + 128, :])
                else:
                    # piece = half: 1024-col block, 2KB/partition
                    cs = slice(piece * h, (piece + 1) * h)
                    eng.dma_start(out=w1_sb[kt][:, cs],
                                  in_=w1t_d[kt * 128:(kt + 1) * 128, cs])

            wc_sb = pw.tile([128, E_TILES, C], bf16, tag="wc", name="wc")

            feats = [pw.tile([128, S_LOC], f32, tag=f"fs_{e}", name=f"fs_{e}")
                     for e in range(E_TILES)]

            def lhsT(kt, e):
                if mode == "fp8":
                    return w1_sb[kt][:, :, e * 128:(e + 1) * 128]
                return w1_sb[kt][:, e * 128:(e + 1) * 128]

            def rhs(xt, kt, w):
                if mode == "fp8":
                    return xt[:, 2 * kt:2 * kt + 2, :w]
                return xt[:, kt, :w]

            featsb = [pw.tile([128, S_LOC], bf16, tag=f"fb_{e}", name=f"fb_{e}")
                      for e in range(E_TILES)]

            def relu_pool(ps, w, e, s0, last=False):
                segs = w // J
                zr = pz.tile([128, CHUNK], f32, tag="zr", name="zr")
                nc.scalar.activation(zr[:, :w], ps[:, :w],
                                     mybir.ActivationFunctionType.Relu)
                nc.vector.tensor_reduce(
                    out=feats[e][:, s0:s0 + segs],
                    in_=zr[:, :w].rearrange("p (s j) -> p s j", j=J),
                    axis=mybir.AxisListType.X,
                    op=mybir.AluOpType.add,
                )
                if last:
                    # feats[e] is complete once the last chunk's pool ran;
                    # convert for MM2 right away so MM2 never waits.
                    nc.vector.tensor_copy(featsb[e], feats[e])

            n_dt = 2 * KT if mode == "fp8" else KT

            # chunk-0 x + W1 startup schedule, two queue heads in parallel
            # (x in d-quads: 1920B descriptors).  wct rides scalar AFTER
            # the startup-critical pieces -- its 4MB would otherwise hog
            # the DMA fabric exactly when chunk 0's weights stream.
            xt0 = px.tile([128, n_dt, CHUNK], in_dt, tag="xt", name="xt")
            cv0 = xt_d[0:D * CHUNK].rearrange("(p d t) -> p d t", p=128, d=n_dt)

            def xquad(eng, q):
                eng.dma_start(out=xt0[:, 4 * q:4 * q + 4, :],
                              in_=cv0[:, 4 * q:4 * q + 4, :])

            # ring bring-up is serialized across queues and costs
            # ~5ns per descriptor of the HEAD DMA, so a big head delays
            # every later ring.  A 1-descriptor micro-DMA per engine
            # brings all three rings up in ~1us total.
            scrs = {}
            for nm, eng in (("sy", nc.sync), ("sc", nc.scalar),
                            ("gp", nc.gpsimd)):
                scr = pw.tile([1, 128], in_dt, tag=f"scr_{nm}",
                              name=f"scr_{nm}")
                eng.dma_start(out=scr, in_=w1t_d[0:1, 0:128])
                scrs[nm] = scr

            if mode == "fp8":
                # x rides gpsimd (its queue aggregates descriptors into
                # ~6KB packets); j-slab weight pieces on sync (j1) and
                # scalar (j0) in consumption order.  kt0's x pair goes
                # alone so fewer bytes queue ahead of kt1's slabs.
                nc.gpsimd.dma_start(out=xt0[:, 0:2, :], in_=cv0[:, 0:2, :])
                nc.gpsimd.dma_start(out=xt0[:, 2:4, :], in_=cv0[:, 2:4, :])
                xquad(nc.gpsimd, 1)      # x for kt2, kt3
                for k in range(KT):
                    wpiece(nc.sync, k, 1)
                for k in range(KT):
                    wpiece(nc.scalar, k, 0)
            else:
                xquad(nc.gpsimd, 0)      # x for kt0..3
                xquad(nc.gpsimd, 1)      # x for kt4..7
                for k in range(KT):
                    wpiece(nc.sync, k, 1)
                for k in range(KT):
                    wpiece(nc.scalar, k, 0)
            # MM2 weights: one big DMA; only needed once MM1 drains.
            nc.scalar.dma_start(out=wc_sb, in_=wct_v)

            with tc.tile_pool(name="ps1", bufs=8, space="PSUM") as pp1:
                # warmup matmuls: ramp the PE p-state while the first real
                # weights/x stream in.  Results are never read.  Shares the
                # "ps" tag so the pool stays at 8 banks.
                wps = pp1.tile([128, CHUNK], f32, tag="ps", name="warm")
                for _ in range(N_WARM):
                    if mode == "fp8":
                        nc.tensor.matmul(wps[:, :WARM_W], dmy[:, :, :128], dmy,
                                         start=True, stop=True, perf_mode=perf)
                    else:
                        nc.tensor.matmul(wps[:, :WARM_W], dmy[:, :128], dmy,
                                         start=True, stop=True)

                for ci, (t0, w) in enumerate(chunks):
                    if ci == 0:
                        xt = xt0
                    else:
                        xt = px.tile([128, n_dt, CHUNK], in_dt, tag="xt",
                                     name="xt")
                        cv = xt_d[D * t0:D * (t0 + w)].rearrange(
                            "(p d t) -> p d t", p=128, d=n_dt)
                        nc.sync.dma_start(out=xt[:, :, :w], in_=cv)
                    s0 = t0 // J
                    if ci == 0:
                        # k-outer across parallel psum groups: first MMs
                        # only need w1_sb[0] + the first x k-slices.
                        e0 = 0
                        for wave in (8, 8):
                            pss = [pp1.tile([128, CHUNK], f32, tag="ps",
                                            name=f"ps0_{e0}_{i}")
                                   for i in range(wave)]
                            for kt in range(KT):
                                for i in range(wave):
                                    nc.tensor.matmul(
                                        pss[i][:, :w],
                                        lhsT(kt, e0 + i),
                                        rhs(xt, kt, w),
                                        start=(kt == 0),
                                        stop=(kt == KT - 1),
                                        perf_mode=perf,
                                    )
                            for i in range(wave):
                                relu_pool(pss[i], w, e0 + i, s0)
                            e0 += wave
                        continue
                    for e in range(E_TILES):
                        ps = pp1.tile([128, CHUNK], f32, tag="ps", name="ps")
                        for kt in range(KT):
                            nc.tensor.matmul(
                                ps[:, :w],
                                lhsT(kt, e),
                                rhs(xt, kt, w),
                                start=(kt == 0),
                                stop=(kt == KT - 1),
                                perf_mode=perf,
                            )
                        relu_pool(ps, w, e, s0, last=(ci == len(chunks) - 1))

                for sb in range(S_LOC // S_BLK):
                    ob = pw.tile([S_BLK, C], bf16, tag=f"ob_{sb}", name=f"ob_{sb}")
                    for c0, cw in ((0, C_BLK), (C_BLK, C_BLK)):
                        ps = pp1.tile([S_BLK, C_BLK], f32, tag="ps", name="ps2")
                        for e in range(E_TILES):
                            nc.tensor.matmul(
                                ps[:, :cw],
                                featsb[e][:, sb * S_BLK:(sb + 1) * S_BLK],
                                wc_sb[:, e, c0:c0 + cw],
                                start=(e == 0),
                                stop=(e == E_TILES - 1),
                            )
                        cs = slice(c0, c0 + cw)
                        r0, r1 = sb * S_BLK, (sb + 1) * S_BLK
                        if sb == 1 and c0 == C_BLK:
                            # final quarter is the critical tail: copy
                            # in two free-dim halves on scalar+vector
                            # concurrently, then DMA as s-halves on two
                            # warm rings so dispatch+completion overlap.
                            mid = c0 + cw // 2
                            nc.scalar.copy(ob[:, c0:mid], ps[:, :cw // 2])
                            nc.vector.tensor_copy(ob[:, mid:c0 + cw],
                                                  ps[:, cw // 2:cw])
                            sh = S_BLK // 2
                            nc.gpsimd.dma_start(out=out_d[r0:r0 + sh, cs],
                                                in_=ob[:sh, cs])
                            nc.scalar.dma_start(out=out_d[r0 + sh:r1, cs],
                                                in_=ob[sh:, cs])
                        else:
                            nc.scalar.copy(ob[:, cs], ps[:, :cw])
                            # per c-chunk, alternating rings: each half
                            # leaves as soon as its copy lands and the
                            # completions overlap.
                            eng = nc.gpsimd if c0 == 0 else nc.scalar
                            eng.dma_start(out=out_d[r0:r1, cs],
                                          in_=ob[:, cs])

    nc.compile()
    return nc


def _prep_inputs(x, W1, Wlin, mode=MODE):
    wc = (Wlin[:, :E] + Wlin[:, E:]) / np.float32(J)     # [C, E] f32
    if mode == "fp8":
        in_np = FP8
        W1 = W1 * np.float32(W1_SCALE)
        wc = wc / np.float32(W1_SCALE)
    else:
        in_np = BF16
    wct = np.ascontiguousarray(wc.T).astype(BF16)        # [E, C] bf16
    w1t = np.ascontiguousarray(W1.T).astype(in_np)       # [D, E]
    in_maps = []
    for c in range(N_CORES):
        xs = x[c * T_LOC:(c + 1) * T_LOC]                # [7500, 1024]
        pieces = []
        t0 = 0
        while t0 < T_LOC:                                # [p][d_tile][t] chunks
            w = min(CHUNK, T_LOC - t0)
            blk = xs[t0:t0 + w].T.reshape(8, 128, w).transpose(1, 0, 2)
            pieces.append(np.ascontiguousarray(blk).astype(in_np).ravel())
            t0 += w
        xt = np.concatenate(pieces)                      # [D*T_LOC] flat
        in_maps.append({"xt": xt, "w1t": w1t, "wct": wct})
    return in_maps


def _run(in_maps, mode=MODE, trace=False, **kw):
    from concourse.bass_utils import run_bass_kernel_spmd

    if mode not in _cache:
        _cache[mode] = _build(mode)
    res = run_bass_kernel_spmd(_cache[mode], in_maps,
                               core_ids=list(range(N_CORES)), trace=trace, **kw)
    logits = np.concatenate(
        [np.asarray(r["out"]) for r in res.results], axis=0).astype(np.float32)
    return logits, res


def kernel(x, y, W1, W2, Wlin):
    x = np.asarray(x, dtype=np.float32)
    W1 = np.asarray(W1, dtype=np.float32)
    Wlin = np.asarray(Wlin, dtype=np.float32)
    modes = (MODE, "bf16") if MODE != "bf16" else ("bf16",)
    for i, mode in enumerate(modes):
        try:
            logits, _ = _run(_prep_inputs(x, W1, Wlin, mode=mode), mode=mode)
            return logits
        except Exception:
            if i == len(modes) - 1:
                raise
    raise RuntimeError("unreachable")


# revision 33
# speedup vs baseline: 1.0175x; 1.0175x over previous
"""Trainium2 Bass kernel for nn_Classifier_69818988363910 (segment_reduce).

Reference computation (after dead-code elimination):
    local = relu(x @ W1^T)                        # [60000, 2048]
    feats = local.reshape(2000, 30, 2048).mean(1) # [2000, 2048]
    logits = concat(feats, feats) @ Wlin^T        # [2000, 1000]
           = feats @ (Wlin[:, :2048] + Wlin[:, 2048:])^T
y / W2 are computed but unused in the reference (original-code bug), so the
output depends only on x, W1, Wlin.

Sharding: data-parallel over the 8 NeuronCores along T (7500 rows = 250
segments per core); W1 / Wc replicated. No collectives; host gathers.

Device kernel per core (fp32 accumulation in PSUM throughout):
    MM1 on PE:   z[e, t] = sum_d W1T[d, e] * xT[d, t]
                 bf16 mode: 8 k-tiles of 128;  fp8 mode: 4 DoubleRow
                 super-k-tiles of 256 (2x PE rate)
    relu on ACT: psum -> sbuf
    pool on DVE: tensor_reduce over [128, segs, 30] view (sum; the 1/30
                 mean scale and the fp8 W1 pre-scale are folded into Wc
                 on the host)
    MM2 on PE:   logits[s, c] = sum_e featsT[e, s] * WcT[e, c]  (bf16)

Perf notes (from trace analysis; baseline 240us -> ~237us):
  - The PE runs MM1 at the fp8-DoubleRow peak (202ns per 256x128x480
    matmul, 157 TF/s/core), so wall time = startup + 202.7us MM1 +
    13.5us MM2 + writeback + ~8us fixed framework teardown.
  - The Tensor engine p-state ramps 1.54 -> 0.83 -> 0.42 ns/cycle with
    continuous busy time; a cold start or mid-stream stall drops it
    back (~2us to recover).  Dummy warmup matmuls on a memset tile keep
    the PE busy from preamble-end until the first real weights land, so
    MM1 opens at full clock.
  - Startup is bound by the DMA descriptor generator (~240 B/ns,
    round-robin one DMA per HWDGE ring, ~12ns per descriptor, so only
    2KB+ descriptors reach full rate).  W1 streams as (kt, j) slabs
    (full rows of [D, E]: 2KB/partition contiguous both sides), j1 on
    sync / j0 on scalar in consumption order; chunk-0 x rides gpsimd
    (whose ring aggregates descriptors into ~6KB packets).  A
    1-descriptor micro-DMA per engine first: ring bring-up is
    serialized and costs ~5ns per descriptor of the ring's HEAD DMA.
  - Wc loads as ONE 4MB DMA on scalar AFTER the startup-critical pieces
    (it is only needed ~200us in, and would hog the fabric).
  - logits copy out as bf16 (half the bytes); each 125x500 quarter is
    DMA'd as soon as its copy lands, alternating gpsimd/scalar rings so
    completions overlap.  Host casts back to f32.
"""

import os

import numpy as np
import ml_dtypes

BF16 = ml_dtypes.bfloat16
FP8 = ml_dtypes.float8_e4m3

MODE = os.environ.get("BASS_KERNEL_MODE", "fp8")    # "bf16" | "fp8"
W1_SCALE = 32.0                                     # fp8 mode: keep W1 out of subnormals
N_WARM = int(os.environ.get("BASS_KERNEL_WARMUP", "22"))
WARM_W = 256                                        # warmup matmul moving dim

N_CORES = 8
T, D, E, C, J = 60000, 1024, 2048, 1000, 30
T_LOC = T // N_CORES          # 7500 rows per core
S_LOC = T_LOC // J            # 250 segments per core
CHUNK = 480                   # t-chunk (16 segments); last chunk is 300
E_TILES = E // 128            # 16
S_BLK = 125                   # MM2 output rows per block (2 blocks)
C_BLK = 500                   # MM2 output cols per chunk (2 chunks)

_cache = {}


def _build(mode):
    from concourse import bacc, mybir
    from concourse.tile import TileContext

    f32 = mybir.dt.float32
    bf16 = mybir.dt.bfloat16
    fp8 = mybir.dt.float8e4
    in_dt = fp8 if mode == "fp8" else bf16
    KT = 4 if mode == "fp8" else 8          # accumulation steps per psum group
    perf = mybir.MatmulPerfMode.DoubleRow if mode == "fp8" else None

    nc = bacc.Bacc(trn_type="TRN2", target_bir_lowering=False, debug=False,
                   num_devices=N_CORES, num_swdge_queues=4)

    # x shard pre-chunked on the host to [chunk][p=128][d_tile=8][t] so each
    # chunk is one partition-contiguous DMA (3840B descriptors instead of
    # 8 DMAs x 480B descriptors).  W1 likewise host-packed into
    # [kt][half][p][(j)][e] piece blocks so each (kt, half) piece is one
    # contiguous 2KB run per partition — the HWDGE queue head dispatches
    # descriptors serially (~12-18ns each), so 2KB descriptors move ~4x
    # the bytes/s of the 512B-1KB ones a [D, E] layout would give.
    xt_d = nc.declare_dram_parameter("xt", [D * T_LOC], in_dt, isOutput=False)
    w1t_d = nc.declare_dram_parameter("w1t", [D, E], in_dt, isOutput=False)
    wct_d = nc.declare_dram_parameter("wct", [E, C], bf16, isOutput=False)
    out_d = nc.declare_dram_parameter("out", [S_LOC, C], bf16, isOutput=True)

    # t-chunks: 15 x 480 + 1 x 300
    chunks = []
    t0 = 0
    while t0 < T_LOC:
        w = min(CHUNK, T_LOC - t0)
        chunks.append((t0, w))
        t0 += w

    wct_v = wct_d[:, :].rearrange("(e p) c -> p e c", p=128)
    h = E // 2

    with TileContext(nc) as tc:
        with (
            tc.tile_pool(name="xin", bufs=2) as px,
            tc.tile_pool(name="wgt", bufs=1) as pw,
            tc.tile_pool(name="zrl", bufs=4) as pz,
        ):
            # PE warmup fodder: the memset is gpsimd's first instruction, so
            # the dummy matmuls can start right at preamble-end.
            if mode == "fp8":
                dmy = pw.tile([128, 2, WARM_W], fp8, tag="dmy", name="dmy")
            else:
                dmy = pw.tile([128, WARM_W], bf16, tag="dmy", name="dmy")
            nc.gpsimd.memset(dmy, 0)

            # --- W1 tiles: the HWDGE queue head dispatches descriptors
            # serially (~12ns each), so startup weight latency is set by
            # descriptor SIZE and by how many pieces queue ahead.  Pieces
            # are k-tile halves (1KB descriptors in the [D, E] layout) and
            # ride TWO queue heads in parallel (scalar + sync), interleaved
            # with chunk 0's x k-pairs in exact consumption order:
            #   sync:   x(kt0) w(1,h0) x(kt1) w(3,h0) w(1,h1) w(3,h1) chunks
            #   scalar: w(0,h0) x(kt2) w(2,h0) x(kt3) w(0,h1) w(2,h1)
            # (chunk 0 runs two 8-e-tile waves, kt-outer: wave 1 reads the
            # h0 half of every k-tile, wave 2 the h1 half.)
            if mode == "fp8":
                w1_sb = [pw.tile([128, 2, E], fp8, tag=f"w1_{k}",
                                 name=f"w1_{k}") for k in range(KT)]
            else:
                w1_sb = [pw.tile([128, E], bf16, tag=f"w1_{k}",
                                 name=f"w1_{k}") for k in range(KT)]

            def wpiece(eng, kt, piece):
                if mode == "fp8":
                    # piece = j: one full 128-row j-slab, 2KB/partition
                    r0 = kt * 256 + piece * 128
                    eng.dma_start(out=w1_sb[kt][:, piece, :],
                                  in_=w1t_d[r0:r0 + 128, :])
                else:
                    # piece = half: 1024-col block, 2KB/partition
                    cs = slice(piece * h, (piece + 1) * h)
                    eng.dma_start(out=w1_sb[kt][:, cs],
                                  in_=w1t_d[kt * 128:(kt + 1) * 128, cs])

            wc_sb = pw.tile([128, E_TILES, C], bf16, tag="wc", name="wc")

            feats = [pw.tile([128, S_LOC], f32, tag=f"fs_{e}", name=f"fs_{e}")
                     for e in range(E_TILES)]

            def lhsT(kt, e):
                if mode == "fp8":
                    return w1_sb[kt][:, :, e * 128:(e + 1) * 128]
                return w1_sb[kt][:, e * 128:(e + 1) * 128]

            def rhs(xt, kt, w):
                if mode == "fp8":
                    return xt[:, 2 * kt:2 * kt + 2, :w]
                return xt[:, kt, :w]

            featsb = [pw.tile([128, S_LOC], bf16, tag=f"fb_{e}", name=f"fb_{e}")
                      for e in range(E_TILES)]

            def relu_pool(ps, w, e, s0, last=False):
                segs = w // J
                zr = pz.tile([128, CHUNK], f32, tag="zr", name="zr")
                nc.scalar.activation(zr[:, :w], ps[:, :w],
                                     mybir.ActivationFunctionType.Relu)
                nc.vector.tensor_reduce(
                    out=feats[e][:, s0:s0 + segs],
                    in_=zr[:, :w].rearrange("p (s j) -> p s j", j=J),
                    axis=mybir.AxisListType.X,
                    op=mybir.AluOpType.add,
                )
                if last:
                    # feats[e] is complete once the last chunk's pool ran;
                    # convert for MM2 right away so MM2 never waits.
                    nc.vector.tensor_copy(featsb[e], feats[e])

            n_dt = 2 * KT if mode == "fp8" else KT

            # chunk-0 x + W1 startup schedule, two queue heads in parallel
            # (x in d-quads: 1920B descriptors).  wct rides scalar AFTER
            # the startup-critical pieces -- its 4MB would otherwise hog
            # the DMA fabric exactly when chunk 0's weights stream.
            xt0 = px.tile([128, n_dt, CHUNK], in_dt, tag="xt", name="xt")
            cv0 = xt_d[0:D * CHUNK].rearrange("(p d t) -> p d t", p=128, d=n_dt)

            def xquad(eng, q):
                eng.dma_start(out=xt0[:, 4 * q:4 * q + 4, :],
                              in_=cv0[:, 4 * q:4 * q + 4, :])

            # ring bring-up is serialized across queues and costs
            # ~5ns per descriptor of the HEAD DMA, so a big head delays
            # every later ring.  A 1-descriptor micro-DMA per engine
            # brings all three rings up in ~1us total.
            scrs = {}
            for nm, eng in (("sy", nc.sync), ("sc", nc.scalar),
                            ("gp", nc.gpsimd)):
                scr = pw.tile([1, 128], in_dt, tag=f"scr_{nm}",
                              name=f"scr_{nm}")
                eng.dma_start(out=scr, in_=w1t_d[0:1, 0:128])
                scrs[nm] = scr

            if mode == "fp8":
                # x rides gpsimd (its queue aggregates descriptors into
                # ~6KB packets); j-slab weight pieces on sync (j1) and
                # scalar (j0) in consumption order.  kt0's x pair goes
                # alone so fewer bytes queue ahead of kt1's slabs.
                nc.gpsimd.dma_start(out=xt0[:, 0:2, :], in_=cv0[:, 0:2, :])
                nc.gpsimd.dma_start(out=xt0[:, 2:4, :], in_=cv0[:, 2:4, :])
                xquad(nc.gpsimd, 1)      # x for kt2, kt3
                for k in range(KT):
                    wpiece(nc.sync, k, 1)
                for k in range(KT):
                    wpiece(nc.scalar, k, 0)
            else:
                xquad(nc.gpsimd, 0)      # x for kt0..3
                xquad(nc.gpsimd, 1)      # x for kt4..7
                for k in range(KT):
                    wpiece(nc.sync, k, 1)
                for k in range(KT):
                    wpiece(nc.scalar, k, 0)
            # MM2 weights: one big DMA; only needed once MM1 drains.
            nc.scalar.dma_start(out=wc_sb, in_=wct_v)

            with tc.tile_pool(name="ps1", bufs=8, space="PSUM") as pp1:
                # warmup matmuls: ramp the PE p-state while the first real
                # weights/x stream in.  Results are never read.  Shares the
                # "ps" tag so the pool stays at 8 banks.
                wps = pp1.tile([128, CHUNK], f32, tag="ps", name="warm")
                for _ in range(N_WARM):
                    if mode == "fp8":
                        nc.tensor.matmul(wps[:, :WARM_W], dmy[:, :, :128], dmy,
                                         start=True, stop=True, perf_mode=perf)
                    else:
                        nc.tensor.matmul(wps[:, :WARM_W], dmy[:, :128], dmy,
                                         start=True, stop=True)

                for ci, (t0, w) in enumerate(chunks):
                    if ci == 0:
                        xt = xt0
                    else:
                        xt = px.tile([128, n_dt, CHUNK], in_dt, tag="xt",
                                     name="xt")
                        cv = xt_d[D * t0:D * (t0 + w)].rearrange(
                            "(p d t) -> p d t", p=128, d=n_dt)
                        nc.sync.dma_start(out=xt[:, :, :w], in_=cv)
                    s0 = t0 // J
                    if ci == 0:
                        # k-outer across parallel psum groups: first MMs
                        # only need w1_sb[0] + the first x k-slices.
                        e0 = 0
                        for wave in (8, 8):
                            pss = [pp1.tile([128, CHUNK], f32, tag="ps",
                                            name=f"ps0_{e0}_{i}")
                                   for i in range(wave)]
                            for kt in range(KT):
                                for i in range(wave):
                                    nc.tensor.matmul(
                                        pss[i][:, :w],
                                        lhsT(kt, e0 + i),
                                        rhs(xt, kt, w),
                                        start=(kt == 0),
                                        stop=(kt == KT - 1),
                                        perf_mode=perf,
                                    )
                            for i in range(wave):
                                relu_pool(pss[i], w, e0 + i, s0)
                            e0 += wave
                        continue
                    for e in range(E_TILES):
                        ps = pp1.tile([128, CHUNK], f32, tag="ps", name="ps")
                        for kt in range(KT):
                            nc.tensor.matmul(
                                ps[:, :w],
                                lhsT(kt, e),
                                rhs(xt, kt, w),
                                start=(kt == 0),
                                stop=(kt == KT - 1),
                                perf_mode=perf,
                            )
                        relu_pool(ps, w, e, s0, last=(ci == len(chunks) - 1))

                for sb in range(S_LOC // S_BLK):
                    ob = pw.tile([S_BLK, C], bf16, tag=f"ob_{sb}", name=f"ob_{sb}")
                    for c0, cw in ((0, C_BLK), (C_BLK, C_BLK)):
                        ps = pp1.tile([S_BLK, C_BLK], f32, tag="ps", name="ps2")
                        for e in range(E_TILES):
                            nc.tensor.matmul(
                                ps[:, :cw],
                                featsb[e][:, sb * S_BLK:(sb + 1) * S_BLK],
                                wc_sb[:, e, c0:c0 + cw],
                                start=(e == 0),
                                stop=(e == E_TILES - 1),
                            )
                        cs = slice(c0, c0 + cw)
                        nc.scalar.copy(ob[:, cs], ps[:, :cw])
                        # writeback per c-chunk, alternating rings: each
                        # half leaves as soon as its copy lands and the
                        # two completions overlap.
                        eng = nc.gpsimd if c0 == 0 else nc.scalar
                        eng.dma_start(
                            out=out_d[sb * S_BLK:(sb + 1) * S_BLK, cs],
                            in_=ob[:, cs])

    nc.compile()
    return nc


def _prep_inputs(x, W1, Wlin, mode=MODE):
    wc = (Wlin[:, :E] + Wlin[:, E:]) / np.float32(J)     # [C, E] f32
    if mode == "fp8":
        in_np = FP8
        W1 = W1 * np.float32(W1_SCALE)
        wc = wc / np.float32(W1_SCALE)
    else:
        in_np = BF16
    wct = np.ascontiguousarray(wc.T).astype(BF16)        # [E, C] bf16
    w1t = np.ascontiguousarray(W1.T).astype(in_np)       # [D, E]
    in_maps = []
    for c in range(N_CORES):
        xs = x[c * T_LOC:(c + 1) * T_LOC]                # [7500, 1024]
        pieces = []
        t0 = 0
        while t0 < T_LOC:                                # [p][d_tile][t] chunks
            w = min(CHUNK, T_LOC - t0)
            blk = xs[t0:t0 + w].T.reshape(8, 128, w).transpose(1, 0, 2)
            pieces.append(np.ascontiguousarray(blk).astype(in_np).ravel())
            t0 += w
        xt = np.concatenate(pieces)                      # [D*T_LOC] flat
        in_maps.append({"xt": xt, "w1t": w1t, "wct": wct})
    return in_maps


def _run(in_maps, mode=MODE, trace=False, **kw):
    from concourse.bass_utils import run_bass_kernel_spmd

    if mode not in _cache:
        _cache[mode] = _build(mode)
    res = run_bass_kernel_spmd(_cache[mode], in_maps,
                               core_ids=list(range(N_CORES)), trace=trace, **kw)
    logits = np.concatenate(
        [np.asarray(r["out"]) for r in res.results], axis=0).astype(np.float32)
    return logits, res


def kernel(x, y, W1, W2, Wlin):
    x = np.asarray(x, dtype=np.float32)
    W1 = np.asarray(W1, dtype=np.float32)
    Wlin = np.asarray(Wlin, dtype=np.float32)
    modes = (MODE, "bf16") if MODE != "bf16" else ("bf16",)
    for i, mode in enumerate(modes):
        try:
            logits, _ = _run(_prep_inputs(x, W1, Wlin, mode=mode), mode=mode)
            return logits
        except Exception:
            if i == len(modes) - 1:
                raise
    raise RuntimeError("unreachable")
